# revision 16
# baseline (speedup 1.0000x reference)
"""Trainium2 Bass kernel for DenseConv2d via Winograd F(2,3) along H.

Conv2d: input (32,128,56,56) f32, weight (256,128,3,3) f32, bias (256,) f32,
stride 1, pad 1, dilation 1 -> output (32,256,56,56) f32.

Data-parallel over batch across 8 NeuronCores (4 images per core). Per core,
the conv uses 1D Winograd F(2,3) on the H axis (direct on W): each tile of 2
output rows needs 4 transformed-input row streams instead of 6 tap rows, so
the PE streams 12 matmuls per 2 rows (4 kyw x 3 kx) instead of direct conv's
9 per row -- a 1.5x cut in tensor-engine columns (94us -> 63us floor).

  V0 = x[2t] - x[2t+2]   V1 = x[2t+1] + x[2t+2]
  V2 = x[2t+2] - x[2t+1] V3 = x[2t+1] - x[2t+3]         (DVE, bf16 4x mode)
  M[kyw] = sum_kx U[kyw,kx]^T @ V[kyw] shifted by kx    (PE, PSUM per kyw)
  out[2t]   = M0 + M1 + M2 + b                          (DVE/Pool drain)
  out[2t+1] = M1 - M2 - M3 + b

U[kyw] are the G-transformed weights (host-side, exact in bf16: /2 only).
The drain is 4 fused ops per block via scalar_tensor_tensor:
  t = (M1+b) - M2 ; out_odd = t - M3 ; u = 2*M2 + t ; out_even = u + M0
split 5:3 over th between DVE and GpSimd so each block's drain (~2us) hides
under the next block's 12-matmul fill (2.24us). Blocks of 8 H-tiles use 4
PSUM banks (448 f32 = 1 bank per kyw), double-buffered across all 8 banks.
Outputs store as bf16 (halves store DMA); host upcasts to f32.
"""

import sys

if "/opt/trn_rl_repo" not in sys.path:
    sys.path.insert(0, "/opt/trn_rl_repo")

import numpy as np

N_CORES = 8
N, CI, H, W = 32, 128, 56, 56
CO, KH, KW = 256, 3, 3
NP_CORE = N // N_CORES          # images per core
HP, WP = H + 2, W + 2           # padded spatial dims
COT = CO // 128                 # out-channel tiles of 128
NT = H // 2                     # 28 Winograd H-tiles per image
KYW = 4                         # Winograd input/transform positions
N_WARMUP = 6                    # full-width PE warmup matmuls
N_WARMUP_SMALL = 4              # half-width tail warmups

# th-blocks per (img, cot) pass: sizes and drain split (DVE rows : Pool rows)
BLOCKS = [(0, 8), (8, 8), (16, 8), (24, 4)]
DVE_ROWS = {8: 5, 4: 3}         # leading th rows drained by DVE; rest Pool

_CACHE = {}


def _build_program():
    import concourse.mybir as mybir
    from concourse import bacc
    from concourse.tile import TileContext

    nc = bacc.Bacc(None, target_bir_lowering=False)

    x_d = nc.dram_tensor("x", [CI, NP_CORE, HP, WP], mybir.dt.bfloat16,
                         kind="ExternalInput")
    w_d = nc.dram_tensor("w", [CI, COT, KYW, KW, 128], mybir.dt.bfloat16,
                         kind="ExternalInput")
    b_d = nc.dram_tensor("b2", [128, COT], mybir.dt.float32,
                         kind="ExternalInput")
    y_d = nc.dram_tensor("y", [COT, 128, NP_CORE, H, W], mybir.dt.bfloat16,
                         kind="ExternalOutput")

    f32 = mybir.dt.float32
    bf16 = mybir.dt.bfloat16
    ADD = mybir.AluOpType.add
    SUB = mybir.AluOpType.subtract
    MULT = mybir.AluOpType.mult

    with TileContext(nc) as tc:
        with (
            tc.tile_pool(name="xin", bufs=1) as xpool,
            tc.tile_pool(name="vpool", bufs=1) as vpool,
            tc.tile_pool(name="wpool", bufs=1) as wpool,
            tc.tile_pool(name="bpool", bufs=1) as bpool,
            tc.tile_pool(name="tpool", bufs=2) as tpool,
            tc.tile_pool(name="psum", bufs=8, space="PSUM") as ppool,
            tc.tile_pool(name="out", bufs=4) as opool,
        ):
            # PE warmup on scratch data, concurrent with the first input
            # DMAs: bridges PE-free (post-preamble) to data-ready so the
            # clock-gate window keeps the PE at full speed.
            scratch = xpool.tile([CI, 448], bf16, tag="scratch")
            nc.gpsimd.memset(scratch, 0.0)
            wups = ppool.tile([128, 448], f32, tag="ps")
            for _ in range(N_WARMUP):
                nc.tensor.matmul(wups, scratch[:, 0:128], scratch,
                                 start=True, stop=True)
            for _ in range(N_WARMUP_SMALL):
                nc.tensor.matmul(wups[:, 0:224], scratch[:, 0:128],
                                 scratch[:, 0:224], start=True, stop=True)

            # Weights (already G-transformed host-side) + bias.
            wt = wpool.tile([CI, COT, KYW, KW, 128], bf16, tag="wt")
            bt = bpool.tile([128, COT], f32)

            # Input images; img0 lands as two chunks so block0's V rows are
            # ready early (rows 0:18 cover th 0..7).
            xt = {}
            for img in range(NP_CORE):
                xt[img] = xpool.tile([CI, HP, WP], bf16, tag=f"x{img}",
                                     name=f"x{img}")
            nc.sync.dma_start(out=xt[0][:, 0:18, :], in_=x_d[:, 0, 0:18, :])
            nc.scalar.dma_start(out=wt[:, 0], in_=w_d[:, 0])
            nc.sync.dma_start(out=xt[0][:, 18:HP, :], in_=x_d[:, 0, 18:HP, :])
            nc.scalar.dma_start(out=bt, in_=b_d[:, :])
            nc.scalar.dma_start(out=wt[:, 1], in_=w_d[:, 1])
            for img in range(1, NP_CORE):
                eng = nc.sync if img % 2 else nc.scalar
                eng2 = nc.scalar if img % 2 else nc.sync
                eng.dma_start(out=xt[img][:, 0:29, :], in_=x_d[:, img, 0:29, :])
                eng2.dma_start(out=xt[img][:, 29:HP, :],
                               in_=x_d[:, img, 29:HP, :])

            # Winograd input transform on DVE. Strided-row APs run at half
            # DVE rate, so build three CONTIGUOUS row tensors instead and
            # let the PE read strided row views (free for matmul rhs):
            #   D2[r] = x[r] - x[r+2]   (V0 = D2 even rows, V3 = D2 odd)
            #   A[r]  = x[r+1] + x[r+2] (V1 = A even rows)
            #   S[r]  = x[r+2] - x[r+1] (V2 = S even rows)
            # img0 is split at th=8 so block0 starts once x rows 0:18 land.
            vt = {}

            def v_transform(img, th0, th1):
                # rows r in [2*th0, 2*th1-1]; D2/A/S[r] read x[r .. r+3]
                d2v, av, sv = vt[img]
                x = xt[img]
                r0, r1 = 2 * th0, 2 * th1  # [r0, r1)
                nc.vector.tensor_sub(d2v[:, r0:r1, :],
                                     x[:, r0:r1, :],
                                     x[:, r0 + 2:r1 + 2, :])
                nc.vector.tensor_add(av[:, r0:r1, :],
                                     x[:, r0 + 1:r1 + 1, :],
                                     x[:, r0 + 2:r1 + 2, :])
                nc.vector.tensor_sub(sv[:, r0:r1, :],
                                     x[:, r0 + 2:r1 + 2, :],
                                     x[:, r0 + 1:r1 + 1, :])

            def v_rhs(img, kyw, th0, nt, kx):
                # V0=D2 even rows, V1=A even, V2=S even, V3=D2 odd
                d2v, av, sv = vt[img]
                src = (d2v, av, sv, d2v)[kyw]
                r0 = 2 * th0 + (1 if kyw == 3 else 0)
                return src[:, r0:r0 + 2 * nt - 1:2, kx:kx + W]

            def v_tiles(img):
                d2v = vpool.tile([CI, H, WP], bf16, tag=f"d2_{img % 2}",
                                 name=f"d2_{img}")
                av = vpool.tile([CI, H, WP], bf16, tag=f"a_{img % 2}",
                                name=f"a_{img}")
                sv = vpool.tile([CI, H, WP], bf16, tag=f"s_{img % 2}",
                                name=f"s_{img}")
                return (d2v, av, sv)

            vt[0] = v_tiles(0)
            v_transform(0, 0, 8)
            v_transform(0, 8, NT)

            store_q = [nc.sync, nc.scalar]
            nstore = 0
            nblk = 0

            for img in range(NP_CORE):
                for cot in range(COT):
                    for th0, nt in BLOCKS:
                        # Fill: 12 matmuls, kyw order 1,2,3,0 so the drain
                        # chain (needs M1,M2 first, M0 last) starts early.
                        ps = {}
                        for kyw in (1, 2, 3, 0):
                            ps[kyw] = ppool.tile([128, nt, W], f32, tag="ps",
                                                 name=f"ps{kyw}")
                            for kx in range(KW):
                                rhs = v_rhs(img, kyw, th0, nt, kx)
                                nc.tensor.matmul(
                                    ps[kyw], wt[:, cot, kyw, kx, :], rhs,
                                    start=(kx == 0), stop=(kx == KW - 1),
                                )

                        # Drain: out_even = M0+M1+M2+b, out_odd = M1-M2-M3+b.
                        # DVE/ACT ops may read at most ONE PSUM operand and
                        # GpSimd can't touch PSUM at all, so ACT lifts
                        # t=M1+b and c2=M2 to SBUF, GpSimd combines s=t+c2,
                        # d=t-c2, and DVE fuses the remaining PSUM reads:
                        # even = s+M0, odd = d-M3. (d alternates DVE/GpSimd
                        # to balance engine load.)
                        ot = opool.tile([128, 2 * nt, W], bf16, tag="ot")
                        t = tpool.tile([128, nt, W], bf16, tag="t")
                        c2 = tpool.tile([128, nt, W], bf16, tag="c2")
                        s = tpool.tile([128, nt, W], bf16, tag="s")
                        d = tpool.tile([128, nt, W], bf16, tag="d")
                        nc.scalar.add(t, ps[1], bt[:, cot:cot + 1])
                        nc.scalar.copy(c2, ps[2])
                        nc.gpsimd.tensor_add(s, t, c2)
                        deng = nc.vector if nblk % 2 else nc.gpsimd
                        deng.tensor_sub(d, t, c2)
                        nc.vector.tensor_sub(
                            ot[:, 1:2 * nt:2, :], d, ps[3])
                        nc.vector.tensor_add(
                            ot[:, 0:2 * nt:2, :], s, ps[0])
                        nblk += 1

                        store_q[nstore % 2].dma_start(
                            out=y_d[cot, :, img, 2 * th0:2 * (th0 + nt), :],
                            in_=ot)
                        nstore += 1

                    # Next image's V transform rides DVE slack during cot0.
                    if cot == 0 and img + 1 < NP_CORE:
                        vt[img + 1] = v_tiles(img + 1)
                        v_transform(img + 1, 0, NT)

    nc.compile()
    return nc


def prep_in_maps(input, weight, bias):
    """Host-side layout prep -> one in_map per core."""
    import ml_dtypes

    bf = ml_dtypes.bfloat16
    xp = np.pad(input, ((0, 0), (0, 0), (1, 1), (1, 1))).astype(bf)
    # weight [co, ci, ky, kx] -> G-transform ky -> [ci, cot, kyw, kx, cop]
    g = weight.astype(np.float32)
    u = np.empty((KYW, CO, CI, KW), dtype=np.float32)
    u[0] = g[:, :, 0, :]
    u[1] = 0.5 * (g[:, :, 0, :] + g[:, :, 1, :] + g[:, :, 2, :])
    u[2] = 0.5 * (g[:, :, 0, :] - g[:, :, 1, :] + g[:, :, 2, :])
    u[3] = g[:, :, 2, :]
    # [kyw, co, ci, kx] -> [ci, cot, kyw, kx, cop]
    wr = np.ascontiguousarray(
        u.reshape(KYW, COT, 128, CI, KW).transpose(3, 1, 0, 4, 2)
    ).astype(bf)
    b2 = np.ascontiguousarray(bias.reshape(COT, 128).T.astype(np.float32))

    in_maps = []
    for c in range(N_CORES):
        xc = np.ascontiguousarray(
            xp[c * NP_CORE:(c + 1) * NP_CORE].transpose(1, 0, 2, 3))
        in_maps.append({"x": xc, "w": wr, "b2": b2})
    return in_maps


def kernel(input, weight, bias):
    input = np.asarray(input, dtype=np.float32)
    weight = np.asarray(weight, dtype=np.float32)
    bias = np.asarray(bias, dtype=np.float32)

    if "nc" not in _CACHE:
        _CACHE["nc"] = _build_program()
    nc = _CACHE["nc"]

    from concourse.bass_utils import run_bass_kernel_spmd

    in_maps = prep_in_maps(input, weight, bias)
    res = run_bass_kernel_spmd(nc, in_maps, core_ids=list(range(N_CORES)))

    out = np.empty((N, CO, H, W), dtype=np.float32)
    for c in range(N_CORES):
        y = res.results[c]["y"]  # [COT, 128, NP_CORE, H, W] bf16
        out[c * NP_CORE:(c + 1) * NP_CORE] = (
            y.astype(np.float32).transpose(2, 0, 1, 3, 4)
            .reshape(NP_CORE, CO, H, W))
    return out
